# revision 1
# baseline (speedup 1.0000x reference)
"""Trainium2 Bass kernel for a diagonal-SSM layer.

Math (per batch b):
    xn    = layernorm(x[b]) * ln_w + ln_b
    alpha = sigmoid(xn @ Wa.T + ba)        # (T, N)
    u     = xn @ Wb.T + bb                 # (T, N)
    h_t   = alpha_t * h_{t-1} + u_t        # scan over T, diagonal in N
    y     = h @ Wc.T + wcb + D * x[b]

Sharding: 8 cores = 4 batches x 2 halves of the N=1024 state channels.
Each core computes a partial y (its 512-channel half projected through
Wc); the host sums the two halves per batch.  Bias + residual terms are
only applied on the j==0 core (j==1 receives zeros for them).

On-chip layout is feature-major ([d, t] / [n, t]): the host passes
x[b].T pre-tiled per (chunk, partition), so the scan runs as the HW
tensor_tensor_scan along the free (time) axis and all matmuls contract
over the partition dim.  Matmul operands are bf16 (full PE rate on
TRN2; fp32 accumulate in PSUM); the scan carries an fp32 state with
fp32 alpha/u inputs so the recurrence itself adds no rounding.

LayerNorm folding.  The host pre-scales the weights, Wa' = Wa * ln_w,
and precomputes w1 = rowsum(Wa'), c = Wa' @ ln_b + ba (pure weight
preprocessing, no activations involved).  On device, with
S[t] = sum_d x[d,t] and Q[t] = sum_d x[d,t]^2 obtained from matmuls
against an all-ones [128,128] stationary operand (every output
partition receives the sum, so stats arrive broadcast to all
partitions), mu = S/D, rstd = 1/sqrt(Q/D - mu^2 + eps):

    pre[n,t] = (Wa' @ (x*rstd))[n,t] - (mu*rstd)[t] * w1[n] + c[n]

i.e. rstd is applied by pre-scaling the matmul moving operand
(xh = x * rstd), and the mean-correction is a rank-1 fixup.

Pipelining: per-engine instruction order is static, so the emission
order software-pipelines chunks to avoid head-of-line blocking:
    ... G(c)+apply(c), scan(c), [x-load+squares+stats-MMs](c+1),
    [mu..rstd chain + xh](c+1), Y(c)+epilogue(c) ...
"""

import numpy as np

D = 1024          # d_model
N = 1024          # state dim
T = 4096          # sequence length
B = 4             # batch
NH = 512          # state channels per core (N/2)
F = 512           # time-chunk (free dim) per tile
NCHUNK = T // F   # 8
P = 128           # partitions
ND = D // P       # 8 d-tiles
NN = NH // P      # 4 n-tiles
LN_EPS = 1e-5

_cache = {}
_VARIANT = "full"   # timing experiments: "full" | "a" | "ab" | "nostats" | "noscan"


def _mmdt():
    import ml_dtypes
    return ml_dtypes.bfloat16


def _build(reps=1, variant=None):
    variant = variant or _VARIANT
    import concourse.bacc as bacc
    import concourse.tile as tile
    from concourse import mybir

    f32 = mybir.dt.float32
    mmdt = mybir.dt.bfloat16
    AF = mybir.ActivationFunctionType
    OP = mybir.AluOpType

    nc = bacc.Bacc(None, target_bir_lowering=False, debug=False)

    # x pre-tiled on host: xc[c, p, a, t] = x[b].T[a*128+p, c*F+t]
    xc = nc.declare_dram_parameter("xc", [NCHUNK, P, ND, F], mmdt, isOutput=False)
    wa3 = nc.declare_dram_parameter("wa3", [P, ND, NH], mmdt, isOutput=False)
    wb3 = nc.declare_dram_parameter("wb3", [P, ND, NH], mmdt, isOutput=False)
    wc3 = nc.declare_dram_parameter("wc3", [P, NN, D], mmdt, isOutput=False)
    onesp = nc.declare_dram_parameter("onesp", [P, P], mmdt, isOutput=False)
    # packed per-feature vectors, pre-tiled: dv[p, a, v], nv[p, a, v]
    dvecp = nc.declare_dram_parameter("dvecp", [P, ND, 2], f32, isOutput=False)
    nvecp = nc.declare_dram_parameter("nvecp", [P, NN, 4], f32, isOutput=False)
    # y partial, tiled like xc (fp32)
    yc = nc.declare_dram_parameter("yc", [NCHUNK, P, ND, F], f32, isOutput=True)

    with tile.TileContext(nc) as tc:
        with (
            tc.tile_pool(name="wc0", bufs=1) as wc0,
            tc.tile_pool(name="xp", bufs=3) as xp,
            tc.tile_pool(name="sqp", bufs=4) as sqp,
            tc.tile_pool(name="xhp", bufs=3) as xhp,
            tc.tile_pool(name="stp", bufs=3) as stp,
            tc.tile_pool(name="st1", bufs=2) as st1,
            tc.tile_pool(name="aup", bufs=4) as aup,
            tc.tile_pool(name="t1p", bufs=3) as t1p,
            tc.tile_pool(name="hp", bufs=8) as hp,
            tc.tile_pool(name="op_", bufs=4) as op_,
            tc.tile_pool(name="ps_misc", bufs=3, space="PSUM") as ps_misc,
            tc.tile_pool(name="ps_g", bufs=2, space="PSUM") as ps_g,
            tc.tile_pool(name="ps_y", bufs=3, space="PSUM") as ps_y,
        ):
            # ---------------- prologue: constants ----------------
            ones_t = wc0.tile([P, P], mmdt, tag="ones")
            nc.sync.dma_start(ones_t[:], onesp[:])
            eps_t = wc0.tile([P, 1], f32, tag="eps")
            nc.vector.memset(eps_t[:], LN_EPS)
            dv_t = wc0.tile([P, ND, 2], f32, tag="dv")
            nc.sync.dma_start(dv_t[:], dvecp[:])
            nv_t = wc0.tile([P, NN, 4], f32, tag="nv")
            nc.sync.dma_start(nv_t[:], nvecp[:])

            def w1_col(key, nt):
                v = 0 if key == "a" else 1
                return nv_t[:, nt, v : v + 1]

            def c_col(key, nt):
                v = 2 if key == "a" else 3
                return nv_t[:, nt, v : v + 1]

            def x_load(c):
                xt = xp.tile([P, ND, F], mmdt, tag="x")
                nc.sync.dma_start(xt[:], xc[c])
                return xt

            # x0, then weights ordered by first use, x1/x2 interleaved
            x_big = {}
            if reps == 1:
                x_big[0] = x_load(0)
            wa_t = wc0.tile([P, ND, NH], mmdt, tag="wa")
            nc.sync.dma_start(wa_t[:], wa3[:])
            if reps == 1:
                x_big[1] = x_load(1)
                x_big[2] = x_load(2)
            wb_t = wc0.tile([P, ND, NH], mmdt, tag="wb")
            nc.sync.dma_start(wb_t[:], wb3[:])
            wc_t = wc0.tile([P, NN, D], mmdt, tag="wc")
            nc.sync.dma_start(wc_t[:], wc3[:])
            w_t = {"a": wa_t, "b": wb_t}

            # ------------- software-pipelined main loop -------------
            stA = {}
            h_prev = [None] * NN

            def stage_a1(c):
                """x load, S-sum matmuls, squares + Q-sum matmuls."""
                x_t = x_big.pop(c) if c in x_big else x_load(c)
                s_ps = ps_misc.tile([P, F], f32, tag="misc")
                q_ps = ps_misc.tile([P, F], f32, tag="misc")
                if variant == "nostats":
                    nc.vector.memset(s_ps[:], 1.0)
                    nc.vector.memset(q_ps[:], 2.0)
                    stA[c] = (x_t, s_ps, q_ps)
                    return
                for dt in range(ND):
                    nc.tensor.matmul(
                        s_ps[:], ones_t[:], x_t[:, dt, :],
                        start=(dt == 0), stop=(dt == ND - 1),
                    )
                for dt in range(ND):
                    sq = sqp.tile([P, F], mmdt, tag="sq")
                    nc.scalar.activation(sq[:], x_t[:, dt, :], AF.Square)
                    nc.tensor.matmul(
                        q_ps[:], ones_t[:], sq[:],
                        start=(dt == 0), stop=(dt == ND - 1),
                    )
                stA[c] = (x_t, s_ps, q_ps)

            def stage_a2(c):
                """mu/rstd chain + xh (lands early in the DVE queue)."""
                x_t, s_ps, q_ps = stA[c]
                mu = stp.tile([P, F], f32, tag="mu")
                nc.vector.tensor_scalar_mul(mu[:], s_ps[:], 1.0 / D)
                musq = st1.tile([P, F], f32, tag="musq")
                nc.vector.tensor_tensor(musq[:], mu[:], mu[:], op=OP.mult)
                var = st1.tile([P, F], f32, tag="var")
                nc.vector.scalar_tensor_tensor(
                    var[:], q_ps[:], 1.0 / D, musq[:],
                    op0=OP.mult, op1=OP.subtract,
                )
                std = st1.tile([P, F], f32, tag="std")
                nc.scalar.activation(std[:], var[:], AF.Sqrt, bias=eps_t[:])
                rstd = stp.tile([P, F], f32, tag="rstd")
                nc.vector.reciprocal_approx_fast(rstd[:], std[:])
                mursd = stp.tile([P, F], f32, tag="mursd")
                nc.vector.tensor_tensor(mursd[:], mu[:], rstd[:], op=OP.mult)
                rstd_h = stp.tile([P, F], mmdt, tag="rstd_h")
                nc.vector.tensor_copy(rstd_h[:], rstd[:])

                xh_t = []
                for dt in range(ND):
                    xh = xhp.tile([P, F], mmdt, tag=f"xh{dt}")
                    nc.vector.tensor_tensor(
                        xh[:], x_t[:, dt, :], rstd_h[:], op=OP.mult
                    )
                    xh_t.append(xh)
                stA[c] = (x_t, xh_t, mursd)

            def stage_b1(c):
                """Per-n-tile: G matmuls + apply + scan (so each scan waits
                only on its own tile's ACT ops)."""
                nonlocal h_prev
                x_t, xh_t, mursd = stA[c]
                h_t = []
                for nt in range(NN):
                    au = {}
                    for key, func in (("a", AF.Sigmoid), ("b", AF.Identity)):
                        g_ps = ps_g.tile([P, F], f32, tag="g")
                        for dt in range(ND):
                            nc.tensor.matmul(
                                g_ps[:],
                                w_t[key][:, dt, nt * P : (nt + 1) * P],
                                xh_t[dt][:],
                                start=(dt == 0),
                                stop=(dt == ND - 1),
                            )
                        t1 = t1p.tile([P, F], f32, tag="t1")
                        # t1 = mursd*w1 - G = -(G - mu*rstd*w1)
                        nc.vector.scalar_tensor_tensor(
                            t1[:], mursd[:], w1_col(key, nt), g_ps[:],
                            op0=OP.mult, op1=OP.subtract,
                        )
                        o = aup.tile([P, F], f32, tag=f"au{key}")
                        nc.scalar.activation(
                            o[:], t1[:], func, bias=c_col(key, nt), scale=-1.0,
                        )
                        au[key] = o
                    h = hp.tile([P, F], mmdt, tag="h")
                    init = 0.0 if c == 0 else h_prev[nt][:, F - 1 : F]
                    if variant == "noscan":
                        nc.vector.tensor_copy(h[:], au["b"][:])
                    else:
                        nc.vector.tensor_tensor_scan(
                            h[:], au["a"][:], au["b"][:], init,
                            op0=OP.mult, op1=OP.add,
                        )
                    h_t.append(h)
                h_prev = h_t
                stA[c] = (x_t, h_t)

            def stage_b2(c):
                """Y matmuls + epilogue + store (split across both DGE rings)."""
                x_t, h_t = stA.pop(c)
                for half in range(2):
                    ob = op_.tile([P, ND // 2, F], f32, tag="o")
                    for k in range(ND // 2):
                        dt = half * (ND // 2) + k
                        y_ps = ps_y.tile([P, F], f32, tag="y")
                        for nt in range(NN):
                            nc.tensor.matmul(
                                y_ps[:],
                                wc_t[:, nt, dt * P : (dt + 1) * P],
                                h_t[nt][:],
                                start=(nt == 0),
                                stop=(nt == NN - 1),
                            )
                        # ob = (x*D_param + wcb) + y_ps in one custom DVE op
                        nc.vector.affine_then_add(
                            ob[:, k, :], x_t[:, dt, :], y_ps[:],
                            scale=dv_t[:, dt, 0:1], bias=dv_t[:, dt, 1:2],
                        )
                    eng = nc.sync if half == 0 else nc.scalar
                    eng.dma_start(
                        yc[c, :, half * (ND // 2) : (half + 1) * (ND // 2), :],
                        ob[:],
                    )

            def whole_body():
                for c0 in (0, 1):
                    stage_a1(c0)
                    stage_a2(c0)
                for c in range(NCHUNK):
                    if variant != "a":
                        stage_b1(c)
                    if c + 2 < NCHUNK:
                        stage_a1(c + 2)
                    if variant not in ("a", "ab"):
                        stage_b2(c)
                    else:
                        stA.pop(c, None)
                    if c + 2 < NCHUNK:
                        stage_a2(c + 2)

            if reps == 1:
                whole_body()
            else:
                with tc.For_i(0, reps, 1):
                    whole_body()

    nc.compile()
    return nc


def _get_nc():
    if "nc" not in _cache:
        _cache["nc"] = _build()
    return _cache["nc"]


def _prep_in_maps(x, W_alpha_w, W_alpha_b, W_B_w, W_B_b, W_C_w, W_C_b,
                  D_param, ln_w, ln_b):
    mmdt = _mmdt()
    x = np.asarray(x, dtype=np.float32)
    assert x.shape == (B, T, D), x.shape
    wa = np.asarray(W_alpha_w, np.float64)
    wb = np.asarray(W_B_w, np.float64)
    lnw = np.asarray(ln_w, np.float64).reshape(D)
    lnb = np.asarray(ln_b, np.float64).reshape(D)
    # weight-only preprocessing (fold ln_w / ln_b into the projections)
    wa_s = wa * lnw
    wb_s = wb * lnw
    w1a = wa_s.sum(1)
    w1b = wb_s.sum(1)
    ca = wa_s @ lnb + np.asarray(W_alpha_b, np.float64).reshape(N)
    cb = wb_s @ lnb + np.asarray(W_B_b, np.float64).reshape(N)
    nvec = np.stack([w1a, w1b, ca, cb], axis=1).astype(np.float32)  # [N, 4]
    dvec = np.stack([np.asarray(D_param, np.float64).reshape(D),
                     np.asarray(W_C_b, np.float64).reshape(D)], axis=1).astype(np.float32)
    zeros_dvec = np.zeros_like(dvec)
    wc = np.asarray(W_C_w, np.float64)

    def tile_feat(v):
        # [D(or NH), k] -> [P, D//P, k]
        d, k = v.shape
        return np.ascontiguousarray(v.reshape(d // P, P, k).transpose(1, 0, 2))

    def tile_w(wT):
        # [D, M] -> [P, ND, M]
        d, m = wT.shape
        return np.ascontiguousarray(wT.reshape(d // P, P, m).transpose(1, 0, 2))

    ones128 = np.ones((P, P), mmdt)
    in_maps = []
    for core in range(8):
        b, j = core // 2, core % 2
        ns = slice(j * NH, (j + 1) * NH)
        xT = x[b].T  # [D, T]
        # xc[c, p, a, t] = xT[a*P+p, c*F+t]
        xtiled = np.ascontiguousarray(
            xT.reshape(ND, P, NCHUNK, F).transpose(2, 1, 0, 3).astype(mmdt))
        in_maps.append({
            "xc": xtiled,
            "wa3": tile_w(wa_s[ns, :].T.astype(mmdt)),
            "wb3": tile_w(wb_s[ns, :].T.astype(mmdt)),
            "wc3": tile_w(np.ascontiguousarray(wc[:, ns].T).astype(mmdt)),
            "onesp": ones128,
            "dvecp": tile_feat(dvec if j == 0 else zeros_dvec),
            "nvecp": tile_feat(nvec[ns, :]),
        })
    return in_maps


def _combine(results):
    y = np.empty((B, T, D), np.float32)
    for b in range(B):
        yc = results[2 * b]["yc"] + results[2 * b + 1]["yc"]  # [NC, P, ND, F]
        # yT[a*P+p, c*F+t] = yc[c, p, a, t]
        y[b] = yc.transpose(2, 1, 0, 3).reshape(D, T).T
    return y


def kernel(x, W_alpha_w, W_alpha_b, W_B_w, W_B_b, W_C_w, W_C_b, D_param, ln_w, ln_b):
    from concourse.bass_utils import run_bass_kernel_spmd

    in_maps = _prep_in_maps(x, W_alpha_w, W_alpha_b, W_B_w, W_B_b,
                            W_C_w, W_C_b, D_param, ln_w, ln_b)
    nc = _get_nc()
    res = run_bass_kernel_spmd(nc, in_maps, list(range(8)))
    _cache["last_results"] = res
    return _combine(res.results)



# revision 2
# speedup vs baseline: 5.8774x; 5.8774x over previous
"""Trainium2 Bass kernel for a diagonal-SSM layer.

Math (per batch b):
    xn    = layernorm(x[b]) * ln_w + ln_b
    alpha = sigmoid(xn @ Wa.T + ba)        # (T, N)
    u     = xn @ Wb.T + bb                 # (T, N)
    h_t   = alpha_t * h_{t-1} + u_t        # scan over T, diagonal in N
    y     = h @ Wc.T + wcb + D * x[b]

Sharding: 8 cores = 4 batches x 2 halves of the N=1024 state channels.
Each core computes a partial y (its 512-channel half projected through
Wc); the host sums the two bf16 halves per batch.  Bias + residual
terms are only applied on the j==0 core.

Key design points (v2, ~200us/core vs 262us for the all-bf16 v1):
- LayerNorm folded into the weights on host: Wa' = (Wa*ln_w), and the
  MEAN correction is folded too:  W_fix = W' - rowsum(W') . 1^T / D,
  so  W_fix @ x = W'@x - w1*mu  and  W_fix @ (x*rstd) = W'@xh -
  w1*mu*rstd.  No fixup matmuls and no fixup vector ops at all;
  biases c = W'@ln_b + b are applied by the ACT engine.
- The alpha projection runs naked fp8-e4m3 with DoubleRow perf mode
  (contract 256/instruction, 2x bf16 PE rate) on host-quantized
  x8 = e4m3(8*x).  Measured on HW: DR instr time == bf16 instr time at
  2x the MACs.  The sigmoid + scan tolerate the ~2.5% fp8 rounding
  (numpy model: rel 0.0052 vs 2e-2 budget); the u and y projections
  must stay bf16 (fp8 there pushes rel err to ~2-5e-2).
- LN stats via fp8-DR ones-matmuls on x8: S = ones@x8,
  Q = ones@Square(x8/8) -> 8 DR instructions/chunk (v1: 16 bf16).
- rstd carried as rstd/4096 so the fp8 result scale (8*512) cancels
  inside the DVE multiply; sqrt gets scale 4096^2 with bias
  4096^2*eps.
- Per-core PE work: 704 matmul instructions (stats 64 DR + A 128 DR +
  B 256 + C 256) ~ 151.5us measured pure-PE floor; total ~200us.
- PSUM rings: stats 2 + A/B groups 3 + Y 3 = 8 banks.  A-PSUM is
  drained by DVE (t1 = PSUM*rstd), B-PSUM by ACT (u = PSUM + cb):
  splitting the consumers across engines measurably beats putting
  both on ACT.
- b2 (Y matmuls + epilogue + store) is emitted one chunk late so the
  PE never waits on the current chunk's scans.
- Output stored bf16 (halves summed on host in fp32).
"""

import numpy as np

D = 1024          # d_model
N = 1024          # state dim
T = 4096          # sequence length
B = 4             # batch
NH = 512          # state channels per core (N/2)
F = 512           # time-chunk (free dim) per tile
NCHUNK = T // F   # 8
P = 128           # partitions
ND = D // P       # 8 d-tiles
NP = ND // 2      # 4 DoubleRow pairs
NN = NH // P      # 4 n-tiles
LN_EPS = 1e-5
SX = 8.0
SW = 512.0
SQ_EPS = (SX * SW) ** 2 * LN_EPS          # 167.77216
SQ_SCALE = (SX * SW) ** 2                 # 16777216.0

_cache = {}


def _build(reps=1):
    import concourse.bacc as bacc
    import concourse.tile as tile
    from concourse import mybir

    f32 = mybir.dt.float32
    bf16 = mybir.dt.bfloat16
    fp8 = mybir.dt.float8e4
    AF = mybir.ActivationFunctionType
    OP = mybir.AluOpType
    DR = mybir.MatmulPerfMode.DoubleRow

    nc = bacc.Bacc(None, target_bir_lowering=False, debug=False)

    # host-tiled inputs
    x8c = nc.declare_dram_parameter("x8c", [NCHUNK, P, NP, 2, F], fp8, isOutput=False)
    xc = nc.declare_dram_parameter("xc", [NCHUNK, P, ND, F], bf16, isOutput=False)
    wa8 = nc.declare_dram_parameter("wa8", [P, NP, 2, NH], fp8, isOutput=False)
    wb3 = nc.declare_dram_parameter("wb3", [P, ND, NH], bf16, isOutput=False)
    wc3 = nc.declare_dram_parameter("wc3", [P, NN, D], bf16, isOutput=False)
    ones8 = nc.declare_dram_parameter("ones8", [P, 2, P], fp8, isOutput=False)
    dvecp = nc.declare_dram_parameter("dvecp", [P, ND, 2], f32, isOutput=False)
    nvecp = nc.declare_dram_parameter("nvecp", [P, NN, 2], f32, isOutput=False)
    yc = nc.declare_dram_parameter("yc", [NCHUNK, P, ND, F], bf16, isOutput=True)

    with tile.TileContext(nc) as tc:
        with (
            tc.tile_pool(name="wc0", bufs=1) as wc0,
            tc.tile_pool(name="xp", bufs=4) as xp,
            tc.tile_pool(name="sqp", bufs=3) as sqp,
            tc.tile_pool(name="xhp", bufs=3) as xhp,
            tc.tile_pool(name="stp", bufs=3) as stp,
            tc.tile_pool(name="st1", bufs=2) as st1,
            tc.tile_pool(name="aup", bufs=4) as aup,
            tc.tile_pool(name="t1p", bufs=3) as t1p,
            tc.tile_pool(name="hp", bufs=8) as hp,
            tc.tile_pool(name="op_", bufs=3) as op_,
            tc.tile_pool(name="ps_misc", bufs=2, space="PSUM") as ps_misc,
            tc.tile_pool(name="ps_g", bufs=3, space="PSUM") as ps_g,
            tc.tile_pool(name="ps_y", bufs=3, space="PSUM") as ps_y,
        ):
            # ---------------- prologue: constants ----------------
            ones_t = wc0.tile([P, 2, P], fp8, tag="ones")
            nc.sync.dma_start(ones_t[:], ones8[:])
            eps_t = wc0.tile([P, 1], f32, tag="eps")
            nc.vector.memset(eps_t[:], SQ_EPS)
            dv_t = wc0.tile([P, ND, 2], f32, tag="dv")
            nc.sync.dma_start(dv_t[:], dvecp[:])
            nv_t = wc0.tile([P, NN, 2], f32, tag="nv")
            nc.sync.dma_start(nv_t[:], nvecp[:])

            def c_col(key, nt):
                v = 0 if key == "a" else 1
                return nv_t[:, nt, v : v + 1]

            def x_load(c):
                x8_t = xp.tile([P, NP, 2, F], fp8, tag="x8", bufs=3)
                nc.sync.dma_start(x8_t[:], x8c[c])
                xt = xp.tile([P, ND, F], bf16, tag="x")
                nc.sync.dma_start(xt[:], xc[c])
                return x8_t, xt

            x_big = {}
            if reps == 1:
                x_big[0] = x_load(0)
            wa_t = wc0.tile([P, NP, 2, NH], fp8, tag="wa")
            nc.sync.dma_start(wa_t[:], wa8[:])
            if reps == 1:
                x_big[1] = x_load(1)
                x_big[2] = x_load(2)
            wb_t = wc0.tile([P, ND, NH], bf16, tag="wb")
            nc.sync.dma_start(wb_t[:], wb3[:])
            wc_t = wc0.tile([P, NN, D], bf16, tag="wc")
            nc.sync.dma_start(wc_t[:], wc3[:])

            # ------------- software-pipelined main loop -------------
            stA = {}
            h_prev = [None] * NN

            def stage_a1(c):
                """x loads, S/Q fp8-DR stats matmuls."""
                x8_t, x_t = x_big.pop(c) if c in x_big else x_load(c)
                s_ps = ps_misc.tile([P, F], f32, tag="misc")
                q_ps = ps_misc.tile([P, F], f32, tag="misc")
                for a in range(NP):
                    nc.tensor.matmul(
                        s_ps[:], ones_t[:], x8_t[:, a, :, :],
                        start=(a == 0), stop=(a == NP - 1), perf_mode=DR,
                    )
                sq = sqp.tile([P, NP, 2, F], fp8, tag="sq")
                for a in range(NP):
                    nc.scalar.activation(sq[:, a, :, :], x8_t[:, a, :, :],
                                         AF.Square, scale=1.0 / SX)
                    nc.tensor.matmul(
                        q_ps[:], ones_t[:], sq[:, a, :, :],
                        start=(a == 0), stop=(a == NP - 1), perf_mode=DR,
                    )
                stA[c] = (x8_t, x_t, s_ps, q_ps)

            def stage_a2(c):
                """mu/rstd chain + xh."""
                x8_t, x_t, s_ps, q_ps = stA[c]
                mu = stp.tile([P, F], f32, tag="mu")
                nc.vector.tensor_scalar_mul(mu[:], s_ps[:], 1.0 / (SX * D))
                musq = st1.tile([P, F], f32, tag="musq")
                nc.vector.tensor_tensor(musq[:], mu[:], mu[:], op=OP.mult)
                var = st1.tile([P, F], f32, tag="var")
                nc.vector.scalar_tensor_tensor(
                    var[:], q_ps[:], 1.0 / D, musq[:],
                    op0=OP.mult, op1=OP.subtract,
                )
                # std' = sqrt(4096^2 var + 4096^2 eps)
                std = st1.tile([P, F], f32, tag="std")
                nc.scalar.activation(std[:], var[:], AF.Sqrt, bias=eps_t[:],
                                     scale=SQ_SCALE)
                rstd = stp.tile([P, F], f32, tag="rstd")       # rstd/4096
                nc.vector.reciprocal_approx_fast(rstd[:], std[:])
                rstd_h = stp.tile([P, 1, F], bf16, tag="rstd_h")  # true rstd
                nc.vector.tensor_scalar_mul(rstd_h[:, 0, :], rstd[:], SX * SW)

                xh_t = []
                for dt in range(ND):
                    xh = xhp.tile([P, F], bf16, tag=f"xh{dt}")
                    nc.vector.tensor_tensor(
                        xh[:], x_t[:, dt, :], rstd_h[:, 0, :], op=OP.mult,
                    )
                    xh_t.append(xh)
                stA[c] = (x8_t, x_t, xh_t, rstd)

            def stage_b1(c):
                """Per n-tile: A fp8-DR + B bf16 matmuls, apply, scan."""
                nonlocal h_prev
                x8_t, x_t, xh_t, rstd = stA[c]
                h_t = []
                for nt in range(NN):
                    ns = slice(nt * P, (nt + 1) * P)
                    # A: fp8 DR, PSUM = 4096*(Wa_fix @ x)
                    a_ps = ps_g.tile([P, F], f32, tag="g")
                    for a in range(NP):
                        nc.tensor.matmul(
                            a_ps[:], wa_t[:, a, :, ns], x8_t[:, a, :, :],
                            start=(a == 0), stop=(a == NP - 1), perf_mode=DR,
                        )
                    t1 = t1p.tile([P, F], f32, tag="t1")
                    nc.vector.tensor_tensor(t1[:], a_ps[:], rstd[:], op=OP.mult)
                    o_a = aup.tile([P, F], f32, tag="aua")
                    nc.scalar.activation(o_a[:], t1[:], AF.Sigmoid,
                                         bias=c_col("a", nt))
                    # B: bf16 on xh
                    b_ps = ps_g.tile([P, F], f32, tag="g")
                    for dt in range(ND):
                        nc.tensor.matmul(
                            b_ps[:], wb_t[:, dt, ns], xh_t[dt][:],
                            start=(dt == 0), stop=(dt == ND - 1),
                        )
                    o_b = aup.tile([P, F], f32, tag="aub")
                    nc.scalar.activation(o_b[:], b_ps[:], AF.Identity,
                                         bias=c_col("b", nt))
                    h = hp.tile([P, F], bf16, tag="h")
                    init = 0.0 if c == 0 else h_prev[nt][:, F - 1 : F]
                    nc.vector.tensor_tensor_scan(
                        h[:], o_a[:], o_b[:], init,
                        op0=OP.mult, op1=OP.add,
                    )
                    h_t.append(h)
                h_prev = h_t
                stA[c] = (x_t, h_t)

            def stage_b2(c):
                """Y matmuls + epilogue + store (split across DMA rings)."""
                x_t, h_t = stA.pop(c)
                for half in range(2):
                    ob = op_.tile([P, ND // 2, F], bf16, tag="o")
                    for k in range(ND // 2):
                        dt = half * (ND // 2) + k
                        y_ps = ps_y.tile([P, F], f32, tag="y")
                        for nt in range(NN):
                            nc.tensor.matmul(
                                y_ps[:],
                                wc_t[:, nt, dt * P : (dt + 1) * P],
                                h_t[nt][:],
                                start=(nt == 0),
                                stop=(nt == NN - 1),
                            )
                        nc.vector.affine_then_add(
                            ob[:, k, :], x_t[:, dt, :], y_ps[:],
                            scale=dv_t[:, dt, 0:1], bias=dv_t[:, dt, 1:2],
                        )
                    eng = nc.sync if half == 0 else nc.scalar
                    eng.dma_start(
                        yc[c, :, half * (ND // 2) : (half + 1) * (ND // 2), :],
                        ob[:],
                    )

            def whole_body():
                # b2 (Y matmuls + epilogue) delayed one chunk so PE never
                # waits on the current chunk's scans.
                for c0 in (0, 1):
                    stage_a1(c0)
                    stage_a2(c0)
                for c in range(NCHUNK):
                    stage_b1(c)
                    if c + 2 < NCHUNK:
                        stage_a1(c + 2)
                        stage_a2(c + 2)
                    if c > 0:
                        stage_b2(c - 1)
                stage_b2(NCHUNK - 1)

            if reps == 1:
                whole_body()
            else:
                with tc.For_i(0, reps, 1):
                    whole_body()

    nc.compile()
    return nc


def _get_nc():
    if "nc" not in _cache:
        _cache["nc"] = _build()
    return _cache["nc"]


def _prep_in_maps(x, W_alpha_w, W_alpha_b, W_B_w, W_B_b, W_C_w, W_C_b,
                  D_param, ln_w, ln_b):
    import ml_dtypes
    bfdt = ml_dtypes.bfloat16
    e4 = ml_dtypes.float8_e4m3

    x = np.asarray(x, dtype=np.float32)
    assert x.shape == (B, T, D), x.shape
    wa = np.asarray(W_alpha_w, np.float64)
    wb = np.asarray(W_B_w, np.float64)
    lnw = np.asarray(ln_w, np.float64).reshape(D)
    lnb = np.asarray(ln_b, np.float64).reshape(D)
    # fold ln into the weights, then fold the mean correction in too
    wa_s = wa * lnw
    wb_s = wb * lnw
    ca = wa_s @ lnb + np.asarray(W_alpha_b, np.float64).reshape(N)
    cb = wb_s @ lnb + np.asarray(W_B_b, np.float64).reshape(N)
    wa_fix = wa_s - wa_s.sum(1, keepdims=True) / D
    wb_fix = wb_s - wb_s.sum(1, keepdims=True) / D

    nvec = np.stack([ca, cb], axis=1).astype(np.float32)          # [N, 2]
    dvec = np.stack([np.asarray(D_param, np.float64).reshape(D),
                     np.asarray(W_C_b, np.float64).reshape(D)], axis=1).astype(np.float32)
    zeros_dvec = np.zeros_like(dvec)
    wc = np.asarray(W_C_w, np.float64)

    def tile_feat(v):
        d, k = v.shape
        return np.ascontiguousarray(v.reshape(d // P, P, k).transpose(1, 0, 2))

    def tile_w(wT):
        # [D, M] -> [P, ND, M]
        d, m = wT.shape
        return np.ascontiguousarray(wT.reshape(d // P, P, m).transpose(1, 0, 2))

    def tile_w8(wT):
        # [D, M] -> [P, NP, 2, M]  (DoubleRow pairs of d-tiles)
        d, m = wT.shape
        return np.ascontiguousarray(
            wT.reshape(NP, 2, P, m).transpose(2, 0, 1, 3))

    ones128 = np.ones((P, 2, P), e4)
    in_maps = []
    x8_all = (SX * x).astype(e4)          # [B, T, D]
    xb_all = x.astype(bfdt)
    for core in range(8):
        b, j = core // 2, core % 2
        ns = slice(j * NH, (j + 1) * NH)
        xT8 = x8_all[b].T                 # [D, T] fp8
        xTb = xb_all[b].T
        # x8c[c, p, a, h, t] = xT8[(2a+h)*P + p, c*F + t]
        x8tiled = np.ascontiguousarray(
            xT8.reshape(NP, 2, P, NCHUNK, F).transpose(3, 2, 0, 1, 4))
        xtiled = np.ascontiguousarray(
            xTb.reshape(ND, P, NCHUNK, F).transpose(2, 1, 0, 3))
        in_maps.append({
            "x8c": x8tiled,
            "xc": xtiled,
            "wa8": tile_w8((SW * wa_fix[ns, :].T).astype(e4)),
            "wb3": tile_w(wb_fix[ns, :].T.astype(bfdt)),
            "wc3": tile_w(np.ascontiguousarray(wc[:, ns].T).astype(bfdt)),
            "ones8": ones128,
            "dvecp": tile_feat(dvec if j == 0 else zeros_dvec),
            "nvecp": tile_feat(nvec[ns, :]),
        })
    return in_maps


def _combine(results):
    y = np.empty((B, T, D), np.float32)
    for b in range(B):
        yc = (results[2 * b]["yc"].astype(np.float32)
              + results[2 * b + 1]["yc"].astype(np.float32))
        y[b] = yc.transpose(2, 1, 0, 3).reshape(D, T).T
    return y


def kernel(x, W_alpha_w, W_alpha_b, W_B_w, W_B_b, W_C_w, W_C_b, D_param, ln_w, ln_b):
    from concourse.bass_utils import run_bass_kernel_spmd

    in_maps = _prep_in_maps(x, W_alpha_w, W_alpha_b, W_B_w, W_B_b,
                            W_C_w, W_C_b, D_param, ln_w, ln_b)
    nc = _get_nc()
    res = run_bass_kernel_spmd(nc, in_maps, list(range(8)))
    _cache["last_results"] = res
    return _combine(res.results)
